# revision 5
# baseline (speedup 1.0000x reference)
# Trainium2 Bass kernel for nn_CrossAttentionBlock (cross-attention block:
# aa<->ligand cross attention + per-side MLP + residual + layernorm).
#
# Sharding: pure data-parallel over batch. B=16 split as 2 batches per core
# across 8 NeuronCores; zero collectives. All weights are replicated.
#
# On-chip strategy (per core, per batch):
#   - activations flow "feature-major" (X^T: [d_model on partitions, tokens
#     free]) so projections are K=128-chunk accumulating matmuls with the
#     weight as the stationary operand and head_dim=128 == partition count.
#   - V and the second MLP linear are computed token-major so the residual
#     add + layernorm reduce over the free dimension.
#   - softmax in [tq, tk] layout (per-partition max/sum); probabilities are
#     PE-transposed (identity matmul) for the attn @ V contraction.
#   - all matmuls run in float32r (full-rate fp32 mode at N>=256).
#   - masking and token-axis bias adds use K=1 matmuls with a ones row,
#     accumulated into the scores/output PSUM tile.
import numpy as np

import concourse.bass as bass
import concourse.mybir as mybir
import concourse.tile as tile
from concourse import bacc
from concourse.bass_utils import run_bass_kernel_spmd

f32 = mybir.dt.float32
f32r = mybir.dt.float32r
AF = mybir.ActivationFunctionType
ALU = mybir.AluOpType
AX = mybir.AxisListType

B, TAA, TLIG, D = 16, 1024, 256, 1024
H, DH, C = 8, 128, 8  # heads, head_dim, feature chunks (D // 128)
NCORES = 8
BL = B // NCORES  # batches per core
C_SCALE = 1.0 / np.sqrt(DH)
NEG_SLOPE = 0.01
EPS = 1e-5
MASK_NEG = -1.0e9


def _build_nc(trace_label: bool = False):
    nc = bacc.Bacc(None, target_bir_lowering=False)

    # ---- DRAM I/O ----
    aaT_d = nc.dram_tensor("aaT", [BL, 128, C, TAA], f32, kind="ExternalInput")
    aa_tok_d = nc.dram_tensor("aa_tok", [BL, TAA, D], f32, kind="ExternalInput")
    ligT_d = nc.dram_tensor("ligT", [BL, 128, C, TLIG], f32, kind="ExternalInput")
    lig_tok_d = nc.dram_tensor("lig_tok", [BL, TLIG, D], f32, kind="ExternalInput")
    mrl_d = nc.dram_tensor("mrl", [BL, TLIG], f32, kind="ExternalInput")
    mra_d = nc.dram_tensor("mra", [BL, TAA], f32, kind="ExternalInput")
    # weights, lhsT form: [p, ot, fc, j] = W[fc*128+p, ot*128+j]
    wT = {
        n: nc.dram_tensor(f"{n}T", [128, C, C, 128], f32, kind="ExternalInput")
        for n in ("Wq", "Wk", "Wo", "Wr1", "Wl1")
    }
    # weights, natural chunk form: [p, fc, o] = W[fc*128+p, o]
    wN = {
        n: nc.dram_tensor(f"{n}N", [128, C, D], f32, kind="ExternalInput")
        for n in ("Wv", "Wr2", "Wl2")
    }
    # biases: partition form [128, C] and row form [1, D]
    bP = {
        n: nc.dram_tensor(f"b_{n}", [128, C], f32, kind="ExternalInput")
        for n in ("bq", "bk", "bo", "br1", "bl1")
    }
    bR = {
        n: nc.dram_tensor(f"br_{n}", [1, D], f32, kind="ExternalInput")
        for n in ("bv", "br2", "bl2")
    }
    gb_d = {
        n: nc.dram_tensor(n, [128, D], f32, kind="ExternalInput")
        for n in ("g_aa_b", "b_aa_b", "g_lig_b", "b_lig_b")
    }
    ones_d = nc.dram_tensor("ones_row", [1, 128], f32, kind="ExternalInput")
    ident_d = nc.dram_tensor("ident", [128, 128], f32, kind="ExternalInput")

    aa_out_d = nc.dram_tensor("aa_out", [BL, TAA, D], f32, kind="ExternalOutput")
    lig_out_d = nc.dram_tensor("lig_out", [BL, TLIG, D], f32, kind="ExternalOutput")
    attn_al_d = nc.dram_tensor(
        "attn_aa_lig", [BL, H, TAA, TLIG], f32, kind="ExternalOutput"
    )
    attn_la_d = nc.dram_tensor(
        "attn_lig_aa", [BL, H, TLIG, TAA], f32, kind="ExternalOutput"
    )

    from contextlib import ExitStack
    with tile.TileContext(nc) as tc, ExitStack() as _st:
        sb = _st.enter_context(tc.tile_pool(name="sb", bufs=1))
        wp = _st.enter_context(tc.tile_pool(name="wp", bufs=3))
        ev = _st.enter_context(tc.tile_pool(name="ev", bufs=3))
        sc = _st.enter_context(tc.tile_pool(name="scal", bufs=4))
        import os as _os
        _mm = int(_os.environ.get("K_MM_BUFS", "2"))
        _tr = int(_os.environ.get("K_TR_BUFS", "2"))
        ps_mm = _st.enter_context(tc.tile_pool(name="ps_mm", bufs=_mm, space="PSUM"))
        ps_sc = _st.enter_context(tc.tile_pool(name="ps_sc", bufs=2, space="PSUM"))
        ps_tr = _st.enter_context(tc.tile_pool(name="ps_tr", bufs=_tr, space="PSUM"))

        # ---- constants (loaded once) ----
        ones_c = sb.tile([1, 128], f32r, name="ones_c", tag="ones")
        nc.sync.dma_start(ones_c[:], ones_d[:].bitcast(f32r))
        ident = sb.tile([128, 128], f32, name="ident", tag="ident")
        nc.sync.dma_start(ident[:], ident_d[:])
        bp_t = {}
        for n in bP:
            bp_t[n] = sb.tile([128, C], f32, name=f"bp_{n}", tag=f"bp_{n}")
            nc.sync.dma_start(bp_t[n][:], bP[n][:])
        br_t = {}
        for n in bR:
            br_t[n] = sb.tile([1, D], f32r, name=f"brt_{n}", tag=f"brt_{n}")
            nc.sync.dma_start(br_t[n][:], bR[n][:].bitcast(f32r))

        def fm_gemm(wT_dram, XT, T, bias_ap, out, func, alpha=0.0):
            """out[:, ot, t] = func((W^T X^T)[o, t] + b[o]); feature-major."""
            nts = (T + 511) // 512
            for ot in range(C):
                w_ot = wp.tile([128, C, 128], f32r, name=f"w_{ot}", tag="w")
                nc.sync.dma_start(w_ot[:], wT_dram[:, ot].bitcast(f32r))
                for ts in range(nts):
                    n = min(512, T - ts * 512)
                    p = ps_mm.tile([128, 512], f32, name="p_mm", tag="mm")[:, :n]
                    for fc in range(C):
                        nc.tensor.matmul(
                            p,
                            w_ot[:, fc],
                            XT[:, fc, ts * 512 : ts * 512 + n],
                            start=(fc == 0),
                            stop=(fc == C - 1),
                        )
                    nc.scalar.activation(
                        out[:, ot, ts * 512 : ts * 512 + n],
                        p,
                        func,
                        bias=bias_ap[:, ot : ot + 1],
                        scale=1.0,
                        alpha=alpha,
                    )

        def tok_gemm(wfull, XT, T, brow, out, func=AF.Copy, alpha=0.0, res_cb=None):
            """token-major: out[:, tt, o] = func(X[t,:] @ W + brow[o]).

            wfull: SBUF [128, C, D] natural-form weight. out: [128, T//128, D].
            res_cb(tt, os, act_ap): optional per-half consumer instead of
            writing to `out`.
            """
            for tt in range(T // 128):
                for os in range(2):
                    p = ps_mm.tile([128, 512], f32, name="p_tok", tag="mm")
                    for fc in range(C):
                        nc.tensor.matmul(
                            p[:],
                            XT[:, fc, tt * 128 : (tt + 1) * 128],
                            wfull[:, fc, os * 512 : (os + 1) * 512],
                            start=(fc == 0),
                            stop=False,
                        )
                    nc.tensor.matmul(
                        p[:],
                        ones_c[:],
                        brow[:, os * 512 : (os + 1) * 512],
                        start=False,
                        stop=True,
                    )
                    if res_cb is not None:
                        res_cb(tt, os, p)
                    else:
                        nc.scalar.activation(
                            out[:, tt, os * 512 : (os + 1) * 512],
                            p[:],
                            func,
                            bias=0.0,
                            scale=1.0,
                            alpha=alpha,
                        )

        def attention(QT, KT, V, mrow, Tq, Tk, attn_out_dram, outT, b):
            """QT [128, C, Tq], KT [128, C, Tk] feature-major (head h = chunk h);
            V [128, Tk//128, D] token-major. outT [128, C, Tq] feature-major.
            attn probabilities are written to attn_out_dram[b, h]."""
            n_qt = Tq // 128
            n_kc = Tk // 128
            n_ts = (Tq + 511) // 512
            for h in range(H):
                attnT = sb.tile(
                    [128, n_kc, Tq], f32r, name="attnT", tag="attnT", bufs=1
                )
                for qt in range(n_qt):
                    p = ps_sc.tile([128, 1024], f32, name="p_sc", tag="sc")[:, :Tk]
                    for os in range((Tk + 511) // 512):
                        n = min(512, Tk - os * 512)
                        nc.tensor.matmul(
                            p[:, os * 512 : os * 512 + n],
                            QT[:, h, qt * 128 : (qt + 1) * 128],
                            KT[:, h, os * 512 : os * 512 + n],
                            start=True,
                            stop=False,
                        )
                        nc.tensor.matmul(
                            p[:, os * 512 : os * 512 + n],
                            ones_c[:],
                            mrow[:, os * 512 : os * 512 + n],
                            start=False,
                            stop=True,
                        )
                    nmx = sc.tile([128, 1], f32, name="nmx", tag="nmx")
                    nc.vector.reduce_max(nmx[:], p[:], axis=AX.X, negate=True)
                    nsc = sc.tile([128, 1], f32, name="nsc", tag="nsc")
                    nc.vector.tensor_scalar_mul(nsc[:], nmx[:], C_SCALE)
                    sme = sc.tile([128, 1], f32, name="sme", tag="sme")
                    att_u = ev.tile([128, Tk], f32, name="att_u", tag="att", bufs=2)
                    nc.scalar.activation(
                        att_u[:],
                        p[:],
                        AF.Exp,
                        bias=nsc[:],
                        scale=C_SCALE,
                        accum_out=sme[:],
                    )
                    rin = sc.tile([128, 1], f32, name="rin", tag="rin")
                    nc.vector.reciprocal(rin[:], sme[:])
                    att_n = ev.tile([128, Tk], f32, name="att_n", tag="att", bufs=2)
                    nc.vector.tensor_scalar_mul(att_n[:], att_u[:], rin[:])
                    nc.sync.dma_start(
                        attn_out_dram[b, h, qt * 128 : (qt + 1) * 128, :], att_n[:]
                    )
                    for c in range(n_kc):
                        tp = ps_tr.tile([128, 128], f32, name="tp", tag="tr")
                        nc.tensor.transpose(
                            tp[:], att_n[:, c * 128 : (c + 1) * 128], ident[:]
                        )
                        nc.vector.tensor_copy(
                            attnT[:, c, qt * 128 : (qt + 1) * 128],
                            tp[:],
                        )
                for ts in range(n_ts):
                    n = min(512, Tq - ts * 512)
                    p = ps_mm.tile([128, 512], f32, name="p_av", tag="mm")[:, :n]
                    for c in range(n_kc):
                        nc.tensor.matmul(
                            p,
                            V[:, c, h * 128 : (h + 1) * 128],
                            attnT[:, c, ts * 512 : ts * 512 + n],
                            start=(c == 0),
                            stop=(c == n_kc - 1),
                        )
                    nc.scalar.activation(
                        outT[:, h, ts * 512 : ts * 512 + n], p, AF.Copy
                    )

        def mlp2_ln(wfull, H1T, T, brow, tok_dram, g_t, b_t, out_dram, b):
            """second MLP linear (token-major) + leaky relu + residual + LN."""
            for tt in range(T // 128):
                res = ev.tile([128, D], f32, name="res", tag="res", bufs=2)

                def cb(tt_, os, p, res=res):
                    lr = ev.tile([128, 512], f32, name="lr", tag="lr")
                    nc.scalar.activation(
                        lr[:], p[:], AF.Lrelu, bias=0.0, scale=1.0, alpha=NEG_SLOPE
                    )
                    rt = ev.tile([128, 512], f32, name="rt", tag="rt")
                    nc.sync.dma_start(
                        rt[:],
                        tok_dram[
                            b,
                            tt_ * 128 : (tt_ + 1) * 128,
                            os * 512 : (os + 1) * 512,
                        ],
                    )
                    nc.vector.tensor_tensor(
                        res[:, os * 512 : (os + 1) * 512], lr[:], rt[:], ALU.add
                    )

                for os in range(2):
                    p = ps_mm.tile([128, 512], f32, name="p_m2", tag="mm")
                    for fc in range(C):
                        nc.tensor.matmul(
                            p[:],
                            H1T[:, fc, tt * 128 : (tt + 1) * 128],
                            wfull[:, fc, os * 512 : (os + 1) * 512],
                            start=(fc == 0),
                            stop=False,
                        )
                    nc.tensor.matmul(
                        p[:],
                        ones_c[:],
                        brow[:, os * 512 : (os + 1) * 512],
                        start=False,
                        stop=True,
                    )
                    cb(tt, os, p)

                ssum = sc.tile([128, 1], f32, name="ssum", tag="ssum")
                nc.vector.reduce_sum(ssum[:], res[:], axis=AX.X)
                nmu = sc.tile([128, 1], f32, name="nmu", tag="nmu")
                nc.vector.tensor_scalar_mul(nmu[:], ssum[:], -1.0 / D)
                mu = sc.tile([128, 1], f32, name="mu", tag="mu")
                nc.vector.tensor_scalar_mul(mu[:], ssum[:], 1.0 / D)
                xsq = ev.tile([128, D], f32, name="xsq", tag="y", bufs=2)
                smq = sc.tile([128, 1], f32, name="smq", tag="smq")
                nc.scalar.activation(
                    xsq[:], res[:], AF.Square, bias=0.0, scale=1.0, accum_out=smq[:]
                )
                msq = sc.tile([128, 1], f32, name="msq", tag="msq")
                nc.vector.tensor_tensor(msq[:], mu[:], mu[:], ALU.mult)
                var = sc.tile([128, 1], f32, name="var", tag="var")
                nc.vector.tensor_scalar_mul(var[:], smq[:], 1.0 / D)
                nc.vector.tensor_tensor(var[:], var[:], msq[:], ALU.subtract)
                nc.vector.tensor_scalar_add(var[:], var[:], EPS)
                std = sc.tile([128, 1], f32, name="std", tag="std")
                nc.scalar.activation(std[:], var[:], AF.Sqrt, bias=0.0, scale=1.0)
                rstd = sc.tile([128, 1], f32, name="rstd", tag="rstd")
                nc.vector.reciprocal(rstd[:], std[:])
                y = ev.tile([128, D], f32, name="y", tag="y", bufs=2)
                nc.vector.tensor_scalar(y[:], res[:], nmu[:], rstd[:], ALU.add, ALU.mult)
                nc.vector.tensor_tensor(y[:], y[:], g_t[:], ALU.mult)
                nc.vector.tensor_tensor(y[:], y[:], b_t[:], ALU.add)
                nc.sync.dma_start(out_dram[b, tt * 128 : (tt + 1) * 128, :], y[:])

        # ================= per-batch pipeline =================
        for b in range(BL):
            # --- load activations (feature-major) + masks ---
            aaT = sb.tile([128, C, TAA], f32r, name="aaT", tag="S1")
            nc.sync.dma_start(aaT[:], aaT_d[b].bitcast(f32r))
            ligT = sb.tile([128, C, TLIG], f32r, name="ligT", tag="T1")
            nc.sync.dma_start(ligT[:], ligT_d[b].bitcast(f32r))
            mrl = sb.tile([1, TLIG], f32r, name="mrl", tag="mrl", bufs=1)
            nc.sync.dma_start(mrl[:], mrl_d[b : b + 1, :].bitcast(f32r))
            mra = sb.tile([1, TAA], f32r, name="mra", tag="mra", bufs=1)
            nc.sync.dma_start(mra[:], mra_d[b : b + 1, :].bitcast(f32r))

            # --- phase A: QKV for attn1 (aa queries over ligand keys) ---
            wvF = sb.tile([128, C, D], f32r, name="wvF", tag="S3")
            nc.sync.dma_start(wvF[:], wN["Wv"][:].bitcast(f32r))
            vlig = sb.tile([128, TLIG // 128, D], f32r, name="vlig", tag="T3")
            tok_gemm(wvF, ligT, TLIG, br_t["bv"], vlig)
            qaaT = sb.tile([128, C, TAA], f32r, name="qaaT", tag="S2")
            fm_gemm(wT["Wq"], aaT, TAA, bp_t["bq"], qaaT, AF.Identity)
            kligT = sb.tile([128, C, TLIG], f32r, name="kligT", tag="T2")
            fm_gemm(wT["Wk"], ligT, TLIG, bp_t["bk"], kligT, AF.Identity)

            # --- phase B: attn1 ---
            o1T = sb.tile([128, C, TAA], f32r, name="o1T", tag="S3")
            attention(qaaT, kligT, vlig, mrl, TAA, TLIG, attn_al_d, o1T, b)

            # --- phase C: fc_out 1 ---
            x1T = sb.tile([128, C, TAA], f32r, name="x1T", tag="S2")
            fm_gemm(wT["Wo"], o1T, TAA, bp_t["bo"], x1T, AF.Identity)

            # --- phase D: MLP1 (aa) ---
            h1T = sb.tile([128, C, TAA], f32r, name="h1T", tag="S3")
            fm_gemm(wT["Wr1"], x1T, TAA, bp_t["br1"], h1T, AF.Lrelu, NEG_SLOPE)

            # --- phase E: MLP2 (aa) + residual + LN -> aa_out ---
            wr2F = sb.tile([128, C, D], f32r, name="wr2F", tag="S2")
            nc.sync.dma_start(wr2F[:], wN["Wr2"][:].bitcast(f32r))
            g_aa = ev.tile([128, D], f32, name="g_aa", tag="gb", bufs=2)
            nc.sync.dma_start(g_aa[:], gb_d["g_aa_b"][:])
            b_aa = ev.tile([128, D], f32, name="b_aa", tag="gb", bufs=2)
            nc.sync.dma_start(b_aa[:], gb_d["b_aa_b"][:])
            mlp2_ln(wr2F, h1T, TAA, br_t["br2"], aa_tok_d, g_aa, b_aa, aa_out_d, b)

            # --- phase F: QKV for attn2 (ligand queries over aa keys) ---
            wvF2 = sb.tile([128, C, D], f32r, name="wvF2", tag="S2")
            nc.sync.dma_start(wvF2[:], wN["Wv"][:].bitcast(f32r))
            vaa = sb.tile([128, TAA // 128, D], f32r, name="vaa", tag="S3")
            tok_gemm(wvF2, aaT, TAA, br_t["bv"], vaa)
            kaaT = sb.tile([128, C, TAA], f32r, name="kaaT", tag="S2")
            fm_gemm(wT["Wk"], aaT, TAA, bp_t["bk"], kaaT, AF.Identity)
            qligT = sb.tile([128, C, TLIG], f32r, name="qligT", tag="T2")
            fm_gemm(wT["Wq"], ligT, TLIG, bp_t["bq"], qligT, AF.Identity)

            # --- phase G: attn2 ---
            o2T = sb.tile([128, C, TLIG], f32r, name="o2T", tag="T3")
            attention(qligT, kaaT, vaa, mra, TLIG, TAA, attn_la_d, o2T, b)

            # --- phase H: fc_out 2 ---
            x2T = sb.tile([128, C, TLIG], f32r, name="x2T", tag="T2")
            fm_gemm(wT["Wo"], o2T, TLIG, bp_t["bo"], x2T, AF.Identity)

            # --- phase I: MLP1 (lig) ---
            h1lT = sb.tile([128, C, TLIG], f32r, name="h1lT", tag="T3")
            fm_gemm(wT["Wl1"], x2T, TLIG, bp_t["bl1"], h1lT, AF.Lrelu, NEG_SLOPE)

            # --- phase J: MLP2 (lig) + residual + LN -> lig_out ---
            wl2F = sb.tile([128, C, D], f32r, name="wl2F", tag="S2")
            nc.sync.dma_start(wl2F[:], wN["Wl2"][:].bitcast(f32r))
            g_lig = ev.tile([128, D], f32, name="g_lig", tag="gb", bufs=2)
            nc.sync.dma_start(g_lig[:], gb_d["g_lig_b"][:])
            b_lig = ev.tile([128, D], f32, name="b_lig", tag="gb", bufs=2)
            nc.sync.dma_start(b_lig[:], gb_d["b_lig_b"][:])
            mlp2_ln(
                wl2F, h1lT, TLIG, br_t["bl2"], lig_tok_d, g_lig, b_lig, lig_out_d, b
            )

    nc.finalize()
    return nc


_NC_CACHE = {}


def _get_nc():
    if "nc" not in _NC_CACHE:
        _NC_CACHE["nc"] = _build_nc()
    return _NC_CACHE["nc"]


def _prep_host(ligand_features, aa_features, mask_l, mask_aa, Wq, bq, Wk, bk, Wv, bv,
               Wo, bo, Wr1, br1, Wr2, br2, Wl1, bl1, Wl2, bl2, g_aa, b_aa, g_lig,
               b_lig):
    a = lambda x: np.ascontiguousarray(np.asarray(x, dtype=np.float32))
    aa = a(aa_features)
    lig = a(ligand_features)
    aaT = np.ascontiguousarray(aa.reshape(B, TAA, C, 128).transpose(0, 3, 2, 1))
    ligT = np.ascontiguousarray(lig.reshape(B, TLIG, C, 128).transpose(0, 3, 2, 1))
    mrl = np.where(np.asarray(mask_l) == 0, np.float32(MASK_NEG), np.float32(0.0))
    mra = np.where(np.asarray(mask_aa) == 0, np.float32(MASK_NEG), np.float32(0.0))
    wT = lambda W: np.ascontiguousarray(
        a(W).reshape(C, 128, C, 128).transpose(1, 2, 0, 3)
    )
    wN = lambda W: np.ascontiguousarray(a(W).reshape(C, 128, D).transpose(1, 0, 2))
    bp = lambda v: np.ascontiguousarray(a(v).reshape(C, 128).T)
    br = lambda v: a(v).reshape(1, D)
    gbb = lambda v: np.ascontiguousarray(np.broadcast_to(a(v), (128, D)))

    common = {
        "WqT": wT(Wq), "WkT": wT(Wk), "WoT": wT(Wo), "Wr1T": wT(Wr1),
        "Wl1T": wT(Wl1), "WvN": wN(Wv), "Wr2N": wN(Wr2), "Wl2N": wN(Wl2),
        "b_bq": bp(bq), "b_bk": bp(bk), "b_bo": bp(bo), "b_br1": bp(br1),
        "b_bl1": bp(bl1), "br_bv": br(bv), "br_br2": br(br2), "br_bl2": br(bl2),
        "g_aa_b": gbb(g_aa), "b_aa_b": gbb(b_aa),
        "g_lig_b": gbb(g_lig), "b_lig_b": gbb(b_lig),
        "ones_row": np.ones((1, 128), np.float32),
        "ident": np.eye(128, dtype=np.float32),
    }
    in_maps = []
    for c in range(NCORES):
        s = slice(c * BL, (c + 1) * BL)
        m = dict(common)
        m["aaT"] = aaT[s]
        m["aa_tok"] = aa[s]
        m["ligT"] = ligT[s]
        m["lig_tok"] = lig[s]
        m["mrl"] = np.ascontiguousarray(mrl[s].astype(np.float32))
        m["mra"] = np.ascontiguousarray(mra[s].astype(np.float32))
        in_maps.append(m)
    return in_maps


def run_on_hw(in_maps, trace=False, **kw):
    nc = _get_nc()
    return run_bass_kernel_spmd(nc, in_maps, list(range(NCORES)), trace=trace, **kw)


def kernel(**inputs):
    in_maps = _prep_host(**inputs)
    res = run_on_hw(in_maps)
    lig_out = np.concatenate([r["lig_out"] for r in res.results], axis=0)
    aa_out = np.concatenate([r["aa_out"] for r in res.results], axis=0)
    attn_al = np.concatenate([r["attn_aa_lig"] for r in res.results], axis=0)
    attn_la = np.concatenate([r["attn_lig_aa"] for r in res.results], axis=0)
    return lig_out, aa_out, attn_al, attn_la
